# revision 1
# baseline (speedup 1.0000x reference)
"""Bass/Trainium2 kernel for a BiLSTM (TF-LSTMCell) cross-entropy loss.

Model (per reference):
  x = emb[inputs]                        # [B,T,E]
  h_fw = LSTM(x, Wk_f, b_f)              # forward over T
  h_bw = reverse(LSTM(reverse(x), Wk_b, b_b))
  logits = concat(h_fw, h_bw) @ W + b    # [B,T,2]
  loss = mean(xent(logits, outputs) * mask)

Sharding: data-parallel over batch. B=64 -> 8 cores x 8 rows.  Each core runs
both LSTM directions (two independent chains interleaved to hide latency),
computes a partial sum of xent*mask; the host sums 8 scalars and divides.

Device layout is feature-major: gate/feature index on the SBUF partition axis,
(time x batch) on the free axis, so per-step elementwise work is [128, small]
and the recurrent matmul keeps the weight stationary (bf16 -> fast weight
load).  z pre-activations accumulate in PSUM in 8-step blocks:
  psum col = l*64 + m*8 + b   (l=step-in-block, m=gate chunk of 128, b=batch)
Gate order is host-permuted to [i, f, o, j] so sigmoid covers one contiguous
[128,48] slice and tanh(j) one [128,16].  x-projection and bias are injected
into each PSUM block ahead of time by large-N matmuls (bias via a K=1
ones-row matmul), keeping the serial chain per step minimal:
  rec-MM (16 bf16 matmuls) -> sigmoid/tanh (ACT) -> cell update (DVE) ->
  tanh(c) (ACT) -> h write (DVE, bf16) -> next rec-MM.
"""

import numpy as np

B, T_FULL, V, E, H = 64, 256, 32000, 256, 256
G = 4 * H            # 1024 gate dim
NCORE = 8
BL = B // NCORE      # 8 batch rows per core
SB = 8               # recurrence steps per PSUM block

_CACHE = {}


def _emit(nc, tc, d, T):
    """Emit the whole kernel under TileContext tc. d = dict of dram handles."""
    from concourse import bass, mybir
    from concourse.masks import make_identity

    f32 = mybir.dt.float32
    bf16 = mybir.dt.bfloat16
    AF = mybir.ActivationFunctionType
    OP = mybir.AluOpType
    X = mybir.AxisListType.X

    NTOK = BL * T
    NTILE = NTOK // 128
    NBLK = T // SB
    L2 = 2 * NTILE       # loss tiles free dim (classes packed)

    persist = tc.alloc_tile_pool(name="persist", bufs=1)

    # ---------------- persistent SBUF buffers ----------------
    idx_sb = persist.tile([128, NTILE], mybir.dt.int32, tag="idx", name="idx")
    ident = persist.tile([128, 128], f32, tag="ident", name="ident")
    xT = persist.tile([128, 2 * NTOK], bf16, tag="xT", name="xT")  # [p, k(2), tok]
    wx = [persist.tile([128, 2048], bf16, tag=f"wx{dd}", name=f"wx{dd}") for dd in range(2)]
    wh = [persist.tile([128, 2048], bf16, tag=f"wh{dd}", name=f"wh{dd}") for dd in range(2)]
    bias = [persist.tile([1, G], f32, tag=f"bias{dd}", name=f"bias{dd}") for dd in range(2)]
    bias16 = [persist.tile([1, G], bf16, tag=f"bias16_{dd}", name=f"bias16_{dd}")
              for dd in range(2)]
    hst = [persist.tile([128, 16 * T], bf16, tag=f"h{dd}", name=f"h{dd}") for dd in range(2)]
    ones64 = persist.tile([1, 64], bf16, tag="ones64", name="ones64")
    ones128 = persist.tile([128, 1], f32, tag="ones128", name="ones128")
    w_out = persist.tile([128, 8], bf16, tag="w_out", name="w_out")
    b_bcast = persist.tile([128, L2], f32, tag="b_bcast", name="b_bcast")
    outs_sb = persist.tile([128, L2], f32, tag="outs", name="outs")
    mask_sb = persist.tile([128, NTILE], f32, tag="mask", name="mask")
    out_sb = persist.tile([1, 1], f32, tag="out_sb", name="out_sb")

    # ---------------- load constants / weights ----------------
    nc.sync.dma_start(idx_sb[:], d["idx"].ap())
    for dd in range(2):
        nc.gpsimd.dma_start(wx[dd][:], d["wx"].ap()[dd])      # f32 -> bf16 cast
        nc.gpsimd.dma_start(wh[dd][:], d["wh"].ap()[dd])
        nc.sync.dma_start(bias[dd][:], d["bias"].ap()[dd : dd + 1])
        # TF LSTMCell forget bias: f-gate rows (permuted order o,i,f,j -> 512:768)
        nc.scalar.add(bias[dd][:, 512:768], bias[dd][:, 512:768], 1.0)
        # tanh(j) = 2*sigmoid(2j)-1: double the j-gate weights and bias so the
        # one big sigmoid op covers j too (x2 is exact in bf16)
        nc.scalar.mul(bias[dd][:, 768:1024], bias[dd][:, 768:1024], 2.0)
        for wt in (wx[dd], wh[dd]):
            for k in range(2):
                nc.vector.tensor_scalar_mul(
                    wt[:, (k * 8 + 6) * 128 : (k * 8 + 8) * 128],
                    wt[:, (k * 8 + 6) * 128 : (k * 8 + 8) * 128], 2.0)
        # bf16 copy for the bias-injection matmuls (fp32 MMs cost 2x on PE)
        nc.vector.tensor_copy(bias16[dd][:], bias[dd][:])
    nc.gpsimd.dma_start(w_out[:], d["wout"].ap())
    nc.sync.dma_start(b_bcast[:], d["bout"].ap())
    nc.sync.dma_start(outs_sb[:], d["outs"].ap())
    nc.sync.dma_start(mask_sb[:], d["mask"].ap())
    make_identity(nc, ident[:])
    nc.gpsimd.memset(ones64[:], 1.0)
    nc.gpsimd.memset(ones128[:], 1.0)

    # ---------------- stage A: gather + transpose ----------------
    xTr = xT[:].rearrange("p (k n) -> p k n", k=2)
    with tc.tile_pool(name="gather", bufs=1) as pg, \
         tc.tile_pool(name="tps", bufs=4, space="PSUM") as pps:
        # interleave from both ends: fw consumes tile 0 first, bw tile NTILE-1
        order = []
        for i in range((NTILE + 1) // 2):
            order.append(i)
            if NTILE - 1 - i > i:
                order.append(NTILE - 1 - i)
        # queue every gather first (enough bufs to run back-to-back), then
        # pipeline the PE transposes + bf16 casts behind them
        xgs = {}
        for i in order:
            xg = pg.tile([128, E], f32, tag=f"xg{i}", name=f"xg{i}")
            xgs[i] = xg
            nc.gpsimd.indirect_dma_start(
                out=xg[:], out_offset=None, in_=d["emb"].ap(),
                in_offset=bass.IndirectOffsetOnAxis(ap=idx_sb[:, i : i + 1], axis=0),
            )
        for i in order:
            for k in range(2):
                ps = pps.tile([128, 128], f32, tag="tp", name="tp")
                nc.tensor.transpose(out=ps[:], in_=xgs[i][:, k * 128 : (k + 1) * 128],
                                    identity=ident[:])
                nc.vector.tensor_copy(xTr[:, k, i * 128 : (i + 1) * 128], ps[:])

    # ---------------- recurrence ----------------
    # hst layout: [p, k(2), t(T), b(8)]  (k-major so the loss-stage stationary
    # operand over tokens is a single contiguous free dim)
    hr = [hst[dd][:].rearrange("p (k t b) -> p k t b", k=2, b=8) for dd in range(2)]
    wxr = [wx[dd][:].rearrange("p (q j) -> p q j", j=128) for dd in range(2)]
    whr = [wh[dd][:].rearrange("p (q j) -> p q j", j=128) for dd in range(2)]

    def t0_of(dd, bi):
        return bi * SB if dd == 0 else T - SB - bi * SB

    zpool = [tc.alloc_tile_pool(name=f"z{dd}", bufs=2, space="PSUM")
             for dd in range(2)]
    ztile = [{}, {}]

    def prefill_ops(dd, bi):
        """Closures emitting x-proj + bias matmuls for block bi of dir dd."""
        zt = zpool[dd].tile([128, SB * 64], f32, tag=f"zt{dd}", name=f"zt{dd}")
        ztile[dd][bi] = zt
        # m-major: col = m*64 + l*8 + b -> x-proj/bias matmuls write contiguous
        # column ranges (strided PSUM out-APs measured ~7x slower per matmul)
        ztr = zt[:].rearrange("p (m l b) -> p m l b", l=SB, m=8, b=8)
        t0 = t0_of(dd, bi)
        # local index of global step s=0 in this block (block 0 only);
        # that region gets no recurrent matmul, so bias must close the group
        l_s0 = None
        if bi == 0:
            l_s0 = (0 - t0) if dd == 0 else (T - 1 - t0)
        ops = []
        for m in range(8):
            for k in range(2):
                def op_x(m=m, k=k):
                    return nc.tensor.matmul(
                        out=ztr[:, m, :, :],
                        lhsT=wxr[dd][:, k * 8 + m, :],
                        rhs=xTr[:, k, t0 * 8 : (t0 + SB) * 8],
                        start=(k == 0), stop=False)
                ops.append(op_x)

            def op_b(m=m, l_s0=l_s0):
                if l_s0 is None:
                    return nc.tensor.matmul(out=ztr[:, m, :, :],
                                     lhsT=bias16[dd][:, m * 128 : (m + 1) * 128],
                                     rhs=ones64[:, 0 : SB * 8],
                                     start=False, stop=False)
                else:
                    rest = slice(1, SB) if l_s0 == 0 else slice(0, SB - 1)
                    nc.tensor.matmul(out=ztr[:, m, rest, :],
                                     lhsT=bias16[dd][:, m * 128 : (m + 1) * 128],
                                     rhs=ones64[:, 0 : (SB - 1) * 8],
                                     start=False, stop=False)
                    return nc.tensor.matmul(out=ztr[:, m, l_s0, :],
                                     lhsT=bias16[dd][:, m * 128 : (m + 1) * 128],
                                     rhs=ones64[:, 0:8],
                                     start=False, stop=True)
            ops.append(op_b)
        return ops

    gp = tc.alloc_tile_pool(name="gates", bufs=6)

    # rolling per-step work tiles: cols 0:64 = sigmoid(gates) [o,i,f,j2]
    # written at step s, cols 64:80 = c written by step s-1.  Fresh pool tile
    # per step keeps every write single-assignment (no per-step cross-engine
    # WAR semaphores on a persistent tile).
    cur_w = [None, None]
    for dd in range(2):
        w0 = gp.tile([128, 80], f32, tag=f"wk{dd}", name=f"wk{dd}")
        nc.gpsimd.memset(w0[:, 64:80], 0.0)
        cur_w[dd] = w0

    def step(dd, s):
        bi = s // SB
        t = s if dd == 0 else T - 1 - s
        l = t - t0_of(dd, bi)
        zt = ztile[dd][bi]
        ztr = zt[:].rearrange("p (m l b) -> p m l b", l=SB, m=8, b=8)
        rec_first = rec_last = None
        if s > 0:
            tp = t - 1 if dd == 0 else t + 1
            for m in range(8):
                for k in range(2):
                    mm = nc.tensor.matmul(out=ztr[:, m, l, :],
                                          lhsT=whr[dd][:, k * 8 + m, :],
                                          rhs=hr[dd][:, k, tp, :],
                                          start=False, stop=(k == 1))
                    rec_last = mm
                    if rec_first is None:
                        rec_first = mm
        w = cur_w[dd]
        nxt = gp.tile([128, 80], f32, tag=f"wk{dd}", name=f"wk{dd}")
        cur_w[dd] = nxt
        # one sigmoid over all four gates [o,i,f,j2]; j-weights were doubled
        # so sig_j2 = sigmoid(2j) and tanh(j) = 2*sig_j2 - 1
        nc.scalar.activation(w[:, 0:64].rearrange("p (m b) -> p m b", b=8),
                             ztr[:, :, l, :], AF.Sigmoid)
        # paired product: [sig_i*sig_j2 | sig_f*c] in one op
        pm = gp.tile([128, 32], f32, tag="pm", name="pm")
        nc.vector.tensor_tensor(pm[:], w[:, 16:48], w[:, 48:80], op=OP.mult)
        # v = 2*sig_i*sig_j2 - sig_i = sig_i * tanh(j)
        vt = gp.tile([128, 16], f32, tag="vt", name="vt")
        nc.vector.scalar_tensor_tensor(out=vt[:], in0=pm[:, 0:16], scalar=2.0,
                                       in1=w[:, 16:32], op0=OP.mult,
                                       op1=OP.subtract)
        # c = sig_f*c + sig_i*tanh(j), written into the NEXT step's work tile
        nc.vector.tensor_tensor(nxt[:, 64:80], vt[:], pm[:, 16:32], op=OP.add)
        tct = gp.tile([128, 16], f32, tag="tct", name="tct")
        nc.scalar.activation(tct[:], nxt[:, 64:80], AF.Tanh)
        nc.vector.tensor_tensor(hr[dd][:, :, t, :],
                                w[:, 0:16].rearrange("p (k b) -> p k b", k=2),
                                tct[:].rearrange("p (k b) -> p k b", k=2),
                                op=OP.mult)
        return rec_first, rec_last

    for op in prefill_ops(0, 0):
        op()
    for op in prefill_ops(1, 0):
        op()
    from concourse.tile_rust import add_dep_helper

    queues = [[], []]
    pending = []
    for s in range(T):
        if s % SB == 0:
            bi = s // SB
            for dd in range(2):
                queues[dd] = prefill_ops(dd, bi + 1) if bi + 1 < NBLK else []
        popped_all = []
        rec_f_first = rec_b_last = None
        for dd in range(2):
            for _ in range(3):
                if queues[dd]:
                    popped_all.append(queues[dd].pop(0)())
            rf, rl = step(dd, s)
            if dd == 0:
                rec_f_first = rf
            else:
                rec_b_last = rl
        # pin prefill into the inter-step PE idle window: after BOTH dirs'
        # recurrent matmuls of this step, before the next step's first
        if rec_f_first is not None:
            for pi in pending:
                add_dep_helper(rec_f_first.ins, pi.ins, sync=False,
                               reason="prefill before next-step rec")
        if rec_b_last is not None:
            for pi in popped_all:
                add_dep_helper(pi.ins, rec_b_last.ins, sync=False,
                               reason="prefill after this-step rec")
            pending = popped_all
        else:
            pending = pending + popped_all
    for dd in range(2):
        for op in queues[dd]:
            op()

    # ---------------- output projection + loss ----------------
    with tc.tile_pool(name="loss", bufs=2) as pl, \
         tc.tile_pool(name="lps", bufs=1, space="PSUM") as plp:
        lg = plp.tile([128, L2], f32, tag="lg", name="lg")
        for ti in range(NTILE):
            for kk in range(4):
                dd, ch = kk // 2, kk % 2
                nc.tensor.matmul(
                    out=lg[:, ti * 2 : ti * 2 + 2],
                    lhsT=hst[dd][:, ch * T * 8 + ti * 128 :
                                  ch * T * 8 + (ti + 1) * 128],
                    rhs=w_out[:, kk * 2 : kk * 2 + 2],
                    start=(kk == 0), stop=(kk == 3))
        logits = pl.tile([128, L2], f32, tag="logits", name="logits")
        nc.vector.tensor_tensor(logits[:], lg[:], b_bcast[:], op=OP.add)
        lr = logits[:].rearrange("p (n l) -> p n l", l=2)
        outr = outs_sb[:].rearrange("p (n l) -> p n l", l=2)
        mx = pl.tile([128, NTILE], f32, tag="mx", name="mx")
        mn = pl.tile([128, NTILE], f32, tag="mn", name="mn")
        nc.vector.tensor_reduce(mx[:], lr, axis=X, op=OP.max)
        nc.vector.tensor_reduce(mn[:], lr, axis=X, op=OP.min)
        dm = pl.tile([128, NTILE], f32, tag="dm", name="dm")
        nc.vector.tensor_tensor(dm[:], mn[:], mx[:], op=OP.subtract)
        # softplus(d) = log1p(e^d) = -ln(sigmoid(-d)), d = mn - mx <= 0
        sg = pl.tile([128, NTILE], f32, tag="sg", name="sg")
        nc.scalar.activation(sg[:], dm[:], AF.Sigmoid, scale=-1.0)
        lsg = pl.tile([128, NTILE], f32, tag="lsg", name="lsg")
        nc.scalar.activation(lsg[:], sg[:], AF.Ln)
        lse = pl.tile([128, NTILE], f32, tag="lse", name="lse")
        nc.vector.tensor_tensor(lse[:], mx[:], lsg[:], op=OP.subtract)
        ol = pl.tile([128, L2], f32, tag="ol", name="ol")
        nc.vector.tensor_tensor(ol[:], logits[:], outs_sb[:], op=OP.mult)
        olr = pl.tile([128, NTILE], f32, tag="olr", name="olr")
        nc.vector.tensor_reduce(olr[:], ol[:].rearrange("p (n l) -> p n l", l=2),
                                axis=X, op=OP.add)
        osum = pl.tile([128, NTILE], f32, tag="osum", name="osum")
        nc.vector.tensor_reduce(osum[:], outr, axis=X, op=OP.add)
        xe = pl.tile([128, NTILE], f32, tag="xe", name="xe")
        nc.vector.tensor_tensor(xe[:], lse[:], osum[:], op=OP.mult)
        nc.vector.tensor_tensor(xe[:], xe[:], olr[:], op=OP.subtract)
        xm = pl.tile([128, NTILE], f32, tag="xm", name="xm")
        xacc = pl.tile([128, 1], f32, tag="xacc", name="xacc")
        nc.vector.scalar_tensor_tensor(out=xm[:], in0=xe[:], scalar=1.0,
                                       in1=mask_sb[:], op0=OP.mult, op1=OP.mult,
                                       accum_out=xacc[:])
        tot = plp.tile([1, 1], f32, tag="tot", name="tot")
        nc.tensor.matmul(out=tot[:], lhsT=xacc[:], rhs=ones128[:],
                         start=True, stop=True)
        nc.scalar.copy(out_sb[:], tot[:])
    nc.sync.dma_start(d["partial"].ap(), out_sb[:])
    gp.release()
    zpool[1].release()
    zpool[0].release()
    persist.release()


def _build(T=T_FULL):
    if T in _CACHE:
        return _CACHE[T]
    from concourse import bacc, mybir, tile

    f32 = mybir.dt.float32
    nc = bacc.Bacc("TRN2", target_bir_lowering=False, debug=False,
                   enable_asserts=False, num_devices=NCORE)
    NTOK = BL * T
    NTILE = NTOK // 128
    d = {
        "idx": nc.dram_tensor("idx", [128, NTILE], mybir.dt.int32,
                              kind="ExternalInput"),
        "emb": nc.dram_tensor("emb", [V, E], f32, kind="ExternalInput"),
        "wx": nc.dram_tensor("wx", [2, 128, 2048], f32, kind="ExternalInput"),
        "wh": nc.dram_tensor("wh", [2, 128, 2048], f32, kind="ExternalInput"),
        "bias": nc.dram_tensor("bias", [2, G], f32, kind="ExternalInput"),
        "wout": nc.dram_tensor("wout", [128, 8], f32, kind="ExternalInput"),
        "bout": nc.dram_tensor("bout", [128, 2 * NTILE], f32,
                               kind="ExternalInput"),
        "outs": nc.dram_tensor("outs", [128, 2 * NTILE], f32,
                               kind="ExternalInput"),
        "mask": nc.dram_tensor("mask", [128, NTILE], f32, kind="ExternalInput"),
        "partial": nc.dram_tensor("partial", [1, 1], f32, kind="ExternalOutput"),
    }
    with tile.TileContext(nc) as tc:
        _emit(nc, tc, d, T)
    nc.compile()
    _CACHE[T] = (nc, d)
    return nc, d


GATE_PERM = np.r_[768:1024, 0:256, 512:768, 256:512]   # [o, i, f, j]


def _stage_core(core, inputs, outputs, mask, emb, Wk_f, b_f, Wk_b, b_b, W, b, T):
    """Build the per-core input map (pure slicing / transposition / layout)."""
    k8 = core * BL
    NTOK = BL * T
    NTILE = NTOK // 128
    idx = np.ascontiguousarray(
        inputs[k8 : k8 + BL, :T].T.reshape(NTOK).reshape(NTILE, 128).T
    ).astype(np.int32)
    wx = np.empty((2, 128, 2048), np.float32)
    wh = np.empty((2, 128, 2048), np.float32)
    bias = np.empty((2, G), np.float32)
    for dd, (Wk, bb) in enumerate(((Wk_f, b_f), (Wk_b, b_b))):
        Wp = Wk[:, GATE_PERM]
        wx[dd] = Wp[:E].reshape(2, 128, 8, 128).transpose(1, 0, 2, 3).reshape(128, 2048)
        wh[dd] = Wp[E:].reshape(2, 128, 8, 128).transpose(1, 0, 2, 3).reshape(128, 2048)
        bias[dd] = bb[GATE_PERM]
    wout = W.reshape(4, 128, 2).transpose(1, 0, 2).reshape(128, 8).astype(np.float32)
    bout = np.tile(b.astype(np.float32), (128, NTILE))
    outs = (outputs[k8 : k8 + BL, :T].transpose(1, 0, 2).reshape(NTOK, 2)
            .reshape(NTILE, 128, 2).transpose(1, 0, 2).reshape(128, 2 * NTILE))
    msk = mask[k8 : k8 + BL, :T].T.reshape(NTOK).reshape(NTILE, 128).T
    return {
        "idx": idx,
        "emb": np.asarray(emb, np.float32),
        "wx": wx, "wh": wh, "bias": bias,
        "wout": wout, "bout": np.ascontiguousarray(bout, dtype=np.float32),
        "outs": np.ascontiguousarray(outs, dtype=np.float32),
        "mask": np.ascontiguousarray(msk, dtype=np.float32),
    }


def run(inputs, outputs, mask, emb, Wk_f, b_f, Wk_b, b_b, W, b,
        T=T_FULL, trace=False):
    from concourse import bass_utils

    nc, d = _build(T)
    args = (np.asarray(inputs), np.asarray(outputs, np.float32),
            np.asarray(mask, np.float32), np.asarray(emb, np.float32),
            np.asarray(Wk_f, np.float32), np.asarray(b_f, np.float32),
            np.asarray(Wk_b, np.float32), np.asarray(b_b, np.float32),
            np.asarray(W, np.float32), np.asarray(b, np.float32))
    in_maps = [_stage_core(kc, *args, T) for kc in range(NCORE)]
    res = bass_utils.run_bass_kernel_spmd(nc, in_maps, core_ids=list(range(NCORE)),
                                          trace=trace)
    total = sum(float(res.results[kc]["partial"][0, 0]) for kc in range(NCORE))
    loss = np.asarray(np.float32(total / (B * T)))
    return loss, res


def kernel(inputs, outputs, mask, emb, Wk_f, b_f, Wk_b, b_b, W, b):
    loss, _ = run(inputs, outputs, mask, emb, Wk_f, b_f, Wk_b, b_b, W, b)
    return loss

